# revision 33
# baseline (speedup 1.0000x reference)
"""BaiChuan attention block (QKV proj + RoPE + causal attention + o_proj) on 8 NeuronCores.

Sharding: tensor-parallel over heads. Each core owns 4 of the 32 heads:
W_pack columns (q/k/v slices) are column-sharded, w_o is row-sharded, and the
8 partial o_proj outputs are summed on the host (cheap f32 reduce) instead of
an on-device all-reduce.

Everything on-device runs in bf16 (fp32 PSUM accumulation). q/k activations
are kept feature-major ("transposed", [feature, batch*seq]) so that softmax
sums run along the PSUM partition axis:
  scoresT[k, q] = K_chunk @ Q_group    (lhsT = KT chunk, rhs = QT group)
  probsT = exp(scoresT * scale)        (scale folded into the ACT scale
                                        operand; no max subtraction:
                                        |scores| <= ~12 for this distribution)
  causal mask  = sliding slice of a constant 0/1 tril tile, applied only to
                 the diagonal chunks, which are also column-trimmed
  outT[d, q]  += V_kd chunk @ probsT   (PSUM accumulate over k chunks)
  denom+bcast  = ones[128x128] @ acc   (one matmul reduces acc over k AND
                                        broadcasts the denominator)
  normalize    = reciprocal_approx_fast + multiply on DVE

V is produced TOKEN-major in phase 1 (vT chains: lhsT = ht tok-chunk, rhs =
w_v columns) so the attention PV matmul's lhsT tiles ([k-tok, d]) load with a
plain contiguous DMA - no DmaTranspose, whose multi-us sequencer hold was
blocking the store queue at the batch seam in the previous version.

Engine assignment: TensorE matmuls; ScalarE = exp + PSUM->SBUF copies + vT
stores; DVE = masks, prob accumulation, normalize, k-ropes (half-width ops so
they never head-of-line-block the mask/acc chain that gates PV matmuls) and
the phase-1 ropes (DVE is idle then); GPSIMD = head q/k/swap loads (SWDGE)
and attention-phase q-ropes; sync HWDGE = phase-1 w/ht loads + q/k stores,
wo loads, v_kd loads, o_proj stores.

Scheduling (engine streams are static and in-order, so overlap is baked into
emission order):
  - t=0 of the qkv projection runs 5 m-chains interleaved by ko-octs so the
    first matmul is gated by ~1.25MB of DMA, not ~5MB.
  - heads (0,0) and (0,1) load+rope DURING phase 1 (idle DVE), into pools
    allocated before the phase-1 pools so addresses never conflict.
  - both batches run as ONE continuous riffle of (b,h,g) j-streams; b=1
    streams weave in as the b=0 streams retire, so there is no batch seam
    where the PE runs dry. A 2-deep scores pipeline keeps >=4 independent
    matmuls between a scores matmul and the PV matmul that needs its probs.
  - o_proj m-chunks and denominator matmuls are queued side work, popped
    after each PV step (denominators with priority: PSUM/acc rings recycle
    only after a group's normalize).
"""

import os
from collections import deque
import numpy as np
import ml_dtypes

import concourse.bass as bass
import concourse.tile as tile
import concourse.mybir as mybir
from concourse import bacc
from concourse.bass_utils import run_bass_kernel_spmd

F32 = mybir.dt.float32
BF16 = mybir.dt.bfloat16
AF = mybir.ActivationFunctionType
BF = ml_dtypes.bfloat16

B, S, H = 2, 2048, 4096
BS = B * S                      # 4096 tokens
D = 128                         # head dim
NCORES = 8
NH_LOC = 4                      # heads per core (32 / 8)
HK = H // 128                   # 32 contraction chunks for qkv proj
M_QK = 2 * NH_LOC               # 8 q/k output row-chunks per core
ST = 512                        # seq tile
NT = BS // ST                   # 8 seq tiles
GP = S // ST                    # 4 q-groups per sequence
SC = S // 128                   # 16 k-chunks per sequence
ROPE_THETA = 10000.0
SCALE = D ** -0.5

# One continuous riffle across BOTH batches: groups of (b, h, g) j-streams
# interleaved. b=1 streams enter as b=0 streams retire (their qkv rows are
# only stored at the end of phase 1), ending group-major so only the last
# group's 32 o_proj chunks drain after the last pv step.
PAIRS = [
    [(0, 0, 1), (0, 0, 2)],
    [(0, 0, 0), (0, 0, 3)],
    [(0, 1, 0), (0, 1, 1)],
    [(0, 2, 0), (0, 1, 2)],
    [(0, 3, 0), (0, 2, 1)],
    [(0, 1, 3), (0, 2, 2)],
    [(0, 3, 1), (0, 2, 3)],
    [(0, 3, 2), (1, 0, 0), (1, 1, 0)],
    [(0, 3, 3), (1, 2, 0), (1, 3, 0)],
    [(1, 0, 1), (1, 1, 1)],
    [(1, 2, 1), (1, 3, 1)],
    [(1, 0, 2), (1, 1, 2)],
    [(1, 2, 2), (1, 3, 2)],
    [(1, 0, 3), (1, 1, 3)],
    [(1, 2, 3), (1, 3, 3)],
]

LAST_RESULT = None              # BassKernelResults of the most recent run (for test.py)


def _riffle(pairs):
    """[(b,h,g)...] stream groups -> flat (b, h, g, j, nj) step list."""
    steps = []
    for grp_list in pairs:
        streams = [[(b, h, g, j, 4 * g + 4) for j in range(4 * g + 4)]
                   for b, h, g in grp_list]
        k = 0
        while any(streams):
            st = streams[k % len(streams)]
            if st:
                steps.append(st.pop(0))
            k += 1
    return steps


def _build_program():
    nc = bacc.Bacc()

    hT = nc.dram_tensor("hT", [H, BS], BF16, kind="ExternalInput")
    w1 = nc.dram_tensor("w1", [128, M_QK, HK, 128], BF16, kind="ExternalInput")
    wv = nc.dram_tensor("wv", [128, HK, 512], BF16, kind="ExternalInput")
    wo = nc.dram_tensor("wo", [NH_LOC * 128, H], BF16, kind="ExternalInput")
    cs = nc.dram_tensor("cs", [128, S], BF16, kind="ExternalInput")
    sn = nc.dram_tensor("sn", [128, S], BF16, kind="ExternalInput")
    maskd = nc.dram_tensor("mask", [128, ST], BF16, kind="ExternalInput")
    out = nc.dram_tensor("out", [H, BS], BF16, kind="ExternalOutput")

    with tile.TileContext(nc) as tc:
        with (
            tc.tile_pool(name="cons", bufs=1) as cons,
            tc.tile_pool(name="dram", bufs=1, space="DRAM") as dram,
            tc.tile_pool(name="ps_acc", bufs=5, space="PSUM") as ps_acc,
            tc.tile_pool(name="ps_sc", bufs=3, space="PSUM") as ps_sc_p,
            tc.tile_pool(name="xload", bufs=1) as xload,
            tc.tile_pool(name="eheads", bufs=1) as eheads,
        ):
            # per-(row-chunk, batch) bounce tiles: a head's read then depends
            # only on the writes that filled its own tile, not on the whole
            # phase-1 write stream.
            qkv_t = [[dram.tile([128, S], BF16, name=f"qkv_{m}_{bb}",
                                tag=f"qkv_{m}_{bb}")
                      for bb in range(B)] for m in range(M_QK)]
            # token-major V bounce: [tok-in-chunk, k-chunk, d] per (b, head)
            vtb = [[dram.tile([128, SC, 128], BF16, name=f"vt_{bb}_{h}",
                              tag=f"vt_{bb}_{h}")
                    for h in range(NH_LOC)] for bb in range(B)]
            hT3 = hT.rearrange("(ko p) s -> p ko s", p=128)

            def emit_qk_load(b, h, pool, tags=("qt", "kt")):
                """Load raw q/k straight into the rope-target tiles (rope
                then runs in place); loads ride the idle GPSIMD SWDGE queue
                so they never head-of-line-block o_proj stores on sync."""
                nm = f"{b}_{h}"
                qt = pool.tile([128, S], BF16, tag=tags[0], name=f"qt_{nm}")
                nc.gpsimd.dma_start(qt[:], qkv_t[h][b][:])
                kt = pool.tile([128, S], BF16, tag=tags[1], name=f"kt_{nm}")
                nc.gpsimd.dma_start(kt[:], qkv_t[NH_LOC + h][b][:])
                return qt, kt

            def emit_swaps(b, h, pool):
                """Partition-swapped q/k copies for the rope sin term."""
                xqs = pool.tile([128, S], BF16, tag="xqs", name=f"xqs_{b}_{h}")
                nc.gpsimd.dma_start(xqs[0:64, :], qkv_t[h][b][64:128, :])
                nc.gpsimd.dma_start(xqs[64:128, :], qkv_t[h][b][0:64, :])
                xks = pool.tile([128, S], BF16, tag="xks", name=f"xks_{b}_{h}")
                nc.gpsimd.dma_start(xks[0:64, :], qkv_t[NH_LOC + h][b][64:128, :])
                nc.gpsimd.dma_start(xks[64:128, :], qkv_t[NH_LOC + h][b][0:64, :])
                return xqs, xks

            def emit_vkd(b, h, pool, tag="vkd", bufs=None):
                """Plain contiguous load of the token-major V tile."""
                v_kd = pool.tile([128, SC, 128], BF16, tag=tag, bufs=bufs,
                                 name=f"vkd_{b}_{h}")
                nc.sync.dma_start(v_kd[:], vtb[b][h][:])
                return v_kd

            def rope_ops_dve(qt, kt, xqs, xks):
                """Six full-width in-place rope emitters for the idle-DVE
                phase-1 heads."""
                return [
                    lambda: nc.vector.tensor_mul(qt[:], qt[:], cs_sb[:]),
                    lambda: nc.vector.tensor_mul(xqs[:], xqs[:], sn_sb[:]),
                    lambda: nc.vector.tensor_add(qt[:], qt[:], xqs[:]),
                    lambda: nc.vector.tensor_mul(kt[:], kt[:], cs_sb[:]),
                    lambda: nc.vector.tensor_mul(xks[:], xks[:], sn_sb[:]),
                    lambda: nc.vector.tensor_add(kt[:], kt[:], xks[:]),
                ]

            def rope_ops_half(qt, kt, xqs, xks):
                """Attention-phase rope on DVE in half-width ops so a rope
                op ahead of a mask/acc op delays it by <=0.7us. (GPSIMD
                tensor ops are NOT an alternative: they steal the SBUF port
                they share with DVE and slow concurrent DVE ops ~10x.)"""
                hs = [slice(0, S // 2), slice(S // 2, S)]
                ops = []
                for dst, src in ((qt, xqs), (kt, xks)):
                    for hsl in hs:
                        ops.append(lambda d=dst, h=hsl: nc.vector.tensor_mul(
                            d[:, h], d[:, h], cs_sb[:, h]))
                    for hsl in hs:
                        ops.append(lambda x=src, h=hsl: nc.vector.tensor_mul(
                            x[:, h], x[:, h], sn_sb[:, h]))
                    for hsl in hs:
                        ops.append(lambda d=dst, x=src, h=hsl: nc.vector.tensor_add(
                            d[:, h], d[:, h], x[:, h]))
                return ops

            # ---------------- Phase 1: qkT = w1.T @ hT ; vT = hT.T @ wv ----------------
            with (
                tc.tile_pool(name="w1p", bufs=1) as w1p,
                tc.tile_pool(name="wvp", bufs=1) as wvp,
                tc.tile_pool(name="htp", bufs=2) as htp,
                tc.tile_pool(name="p1o", bufs=4) as p1o,
            ):
                w_sb = w1p.tile([128, M_QK, HK, 128], BF16, tag="w1")
                wv_sb = wvp.tile([128, HK, 512], BF16, tag="wv")

                # PE clock warmup: the first ~15us are DMA-gated, and the PE
                # p-state only reaches max clock after ~3us of continuous
                # execution. Grind dependency-free matmuls on memset tiles so
                # pass-A starts at full clock instead of ramping through it.
                ones128 = cons.tile([128, 128], BF16, tag="ones128")
                nc.vector.memset(ones128[:], 1.0)
                wm_sb = cons.tile([128, ST], BF16, tag="wm")
                nc.vector.memset(wm_sb[:], 0.0)
                ps_w0 = ps_acc.tile([128, ST], F32, tag="acc", name="ps_warm0")
                for _ in range(12):
                    nc.tensor.matmul(ps_w0[:], ones128[:], wm_sb[:],
                                     start=True, stop=True)

                def emit_ht(t):
                    ht = htp.tile([128, HK, ST], BF16, tag="ht", name=f"ht_{t}")
                    for oct_ in range(4):
                        nc.sync.dma_start(
                            ht[:, oct_ * 8:(oct_ + 1) * 8],
                            hT3[:, oct_ * 8:(oct_ + 1) * 8, t * ST:(t + 1) * ST])
                    return ht

                # Startup: pass-A (t=0, m=0..5) runs ko-oct-interleaved, so
                # ship w/ht in matching oct-sized pieces: the first matmul is
                # gated by w(m0,oct0)+ht(oct0) = 1.25MB, and each later oct's
                # pieces arrive while the PE chews the previous oct. Nothing
                # else (constants included) is emitted ahead of this prefix.
                # 6 concurrent chains (5 acc slots + 1 borrowed from the
                # idle scores ring) give ~10.4us of PE work per oct vs
                # ~7.5us of contended startup DMA - the PE stays ahead.
                NA = 6
                nc.sync.dma_start(w_sb[:, 0, 0:8], w1[:, 0, 0:8])
                ht0 = htp.tile([128, HK, ST], BF16, tag="ht", name="ht_0")
                nc.sync.dma_start(ht0[:, 0:8], hT3[:, 0:8, 0:ST])
                nc.sync.dma_start(w_sb[:, 1:NA, 0:8], w1[:, 1:NA, 0:8])
                for oct_ in range(1, 4):
                    nc.sync.dma_start(ht0[:, oct_ * 8:(oct_ + 1) * 8],
                                      hT3[:, oct_ * 8:(oct_ + 1) * 8, 0:ST])
                    nc.sync.dma_start(w_sb[:, 0:NA, oct_ * 8:(oct_ + 1) * 8],
                                      w1[:, 0:NA, oct_ * 8:(oct_ + 1) * 8])
                nc.sync.dma_start(w_sb[:, NA:M_QK], w1[:, NA:M_QK])
                for oct_ in range(4):
                    nc.sync.dma_start(wv_sb[:, oct_ * 8:(oct_ + 1) * 8],
                                      wv[:, oct_ * 8:(oct_ + 1) * 8])

                # constants ride behind the startup-critical weights: first
                # needed by the t==5 rope hook / first attention scores.
                cs_sb = cons.tile([128, S], BF16, tag="cs")
                nc.scalar.dma_start(cs_sb[:], cs[:])
                sn_sb = cons.tile([128, S], BF16, tag="sn")
                nc.scalar.dma_start(sn_sb[:], sn[:])
                mask_sb = cons.tile([128, ST], BF16, tag="mask")
                nc.scalar.dma_start(mask_sb[:], maskd[:])

                def qk_store(t, m, ob):
                    tc_, tb = t % GP, t // GP
                    # t=7 stores issue from the ACT queue (woven between its
                    # own copies, done pre-transition) so the first b=1 head
                    # loads aren't FIFO-blocked behind them
                    eng = nc.scalar if t == NT - 1 else nc.sync
                    eng.dma_start(qkv_t[m][tb][:, tc_ * ST:(tc_ + 1) * ST], ob[:])

                estaged = {}
                tiles0 = {}
                ht_next = ht0
                for t in range(NT):
                    ht = ht_next
                    if t < NT - 1:
                        # prefetch emission: ht(t+1) enters the sync FIFO
                        # ahead of tile t's qkv stores
                        ht_next = emit_ht(t + 1)
                    ms = list(range(M_QK))
                    if t == 0:
                        psA = [ps_acc.tile([128, ST], F32, tag="acc",
                                           name=f"ps_q_0_{m}") for m in range(5)]
                        psA.append(ps_sc_p.tile([128, ST], F32, tag="sc",
                                                name="ps_q_0_5"))
                        for oct_ in range(4):
                            for m in range(NA):
                                for ko in range(oct_ * 8, oct_ * 8 + 8):
                                    nc.tensor.matmul(
                                        psA[m][:], w_sb[:, m, ko], ht[:, ko],
                                        start=(ko == 0), stop=(ko == HK - 1))
                            if oct_ < 3:
                                # dependency-free fillers bridge the tail of
                                # the next oct's DMA so the clock never gates
                                for _ in range(3):
                                    nc.tensor.matmul(ps_w0[:], ones128[:],
                                                     wm_sb[:], start=True,
                                                     stop=True)
                        for m in range(NA):
                            ob = p1o.tile([128, ST], BF16, tag="ob")
                            nc.scalar.activation(ob[:], psA[m][:], AF.Copy)
                            qk_store(t, m, ob)
                        ms = list(range(NA, M_QK))
                    for m in ms:
                        ps = ps_acc.tile([128, ST], F32, tag="acc",
                                         name=f"ps_q_{t}_{m}")
                        for ko in range(HK):
                            nc.tensor.matmul(
                                ps[:], w_sb[:, m, ko], ht[:, ko],
                                start=(ko == 0), stop=(ko == HK - 1))
                        ob = p1o.tile([128, ST], BF16, tag="ob")
                        nc.scalar.activation(ob[:], ps[:], AF.Copy)
                        qk_store(t, m, ob)
                    # token-major V: out[tok, d] accumulated over ko; lhsT is
                    # the ht token-chunk itself - no transpose anywhere.
                    tc_, tb = t % GP, t // GP
                    for c in range(4):
                        ps = ps_acc.tile([128, ST], F32, tag="acc",
                                         name=f"ps_v_{t}_{c}")
                        for ko in range(HK):
                            nc.tensor.matmul(
                                ps[:], ht[:, ko, c * 128:(c + 1) * 128],
                                wv_sb[:, ko],
                                start=(ko == 0), stop=(ko == HK - 1))
                        obv = p1o.tile([128, ST], BF16, tag="obv")
                        nc.scalar.activation(obv[:], ps[:], AF.Copy)
                        for h in range(NH_LOC):
                            nc.scalar.dma_start(
                                vtb[tb][h][:, tc_ * 4 + c, :],
                                obv[:, h * 128:(h + 1) * 128])
                    if t == 3:
                        # heads (0,0)/(0,2) raw q/k + swaps; their ropes run
                        # on the idle DVE at t=5/6 so early attention carries
                        # only h1/h3/b1 rope work
                        qt0, kt0 = emit_qk_load(0, 0, eheads, tags=("qt0", "kt0"))
                        sw0 = emit_swaps(0, 0, xload)
                        estaged[0] = (qt0, kt0) + sw0
                    elif t == 4:
                        qt2, kt2 = emit_qk_load(0, 2, eheads, tags=("qt2", "kt2"))
                        # xload ring is 1-deep: h2's swap DMAs queue on
                        # gpsimd behind the WAR on h0's t=5 rope reads
                        sw2 = emit_swaps(0, 2, xload)
                        estaged[2] = (qt2, kt2) + sw2
                    elif t == 5:
                        for op in rope_ops_dve(*estaged[0]):
                            op()
                        tiles0[0] = (estaged[0][0], estaged[0][1],
                                     emit_vkd(0, 0, eheads, tag="vk0"))
                    elif t == 6:
                        for op in rope_ops_dve(*estaged[2]):
                            op()
                        tiles0[2] = [estaged[2][0], estaged[2][1], None]

            # ---------------- Phase 2+3: attention with interleaved o_proj ----------------
            with (
                tc.tile_pool(name="headp", bufs=6) as headp,
            tc.tile_pool(name="xatt", bufs=2) as xatt,
                tc.tile_pool(name="probsp", bufs=6) as probsp,
                tc.tile_pool(name="accp", bufs=5) as accp,
                tc.tile_pool(name="stagep", bufs=22) as stagep,
                tc.tile_pool(name="miscp", bufs=2) as miscp,
                tc.tile_pool(name="p3w", bufs=1) as wop,
                tc.tile_pool(name="p3o", bufs=6) as p3o,
            ):
                wo_sb = wop.tile([128, NH_LOC, H], BF16, tag="wo")
                wo3 = wo.rearrange("(ko p) f -> p ko f", p=128)

                def emit_wo_load(q):
                    # quarter loads deferred into hooks so head loads aren't
                    # queued behind 4.2MB of weights; quarter q covers o_proj
                    # chunks m in [8q, 8q+8)
                    def fn():
                        c0 = q * (H // 4)
                        nc.sync.dma_start(wo_sb[:, :, c0:c0 + H // 4],
                                          wo3[:, :, c0:c0 + H // 4])
                    return fn

                # PE side-work queues. norms (denominator matmuls) must never
                # starve behind o_proj chunks: a group's PSUM/acc slots free
                # only after its normalize runs, and the rings wrap quickly.
                norms = deque()     # entries: (emit_fn, pushed_step)
                fills = deque()
                gstep = [0]

                def make_oproj_chunk(t, m, stages):
                    def emit():
                        ps = ps_acc.tile([128, ST], F32, tag="acc", name=f"ps_o_{t}_{m}")
                        for ko in range(NH_LOC):
                            nc.tensor.matmul(
                                ps[:], wo_sb[:, ko, m * 128:(m + 1) * 128],
                                stages[ko][:],
                                start=(ko == 0), stop=(ko == NH_LOC - 1))
                        ob = p3o.tile([128, ST], BF16, tag="ob3", name=f"ob3_{t}_{m}")
                        # copies stay on ScalarE: a measured DVE/ACT split
                        # rebalanced busy% but WORSENED PE idle - DVE (masks/
                        # accs/ropes) is the critical engine, ScalarE is not
                        nc.scalar.activation(ob[:], ps[:], AF.Copy)
                        nc.sync.dma_start(
                            out[m * 128:(m + 1) * 128, t * ST:(t + 1) * ST], ob[:])
                    return emit

                def make_norm(b, h, g, ps_out, acc, stages):
                    def emit():
                        # denominator broadcast rides the "acc" ring so the
                        # scores ring keeps all 3 slots for the 2-deep
                        # scores pipeline
                        ps_bc = ps_acc.tile([128, ST], F32, tag="acc", name=f"ps_bc_{b}_{h}_{g}")
                        nc.tensor.matmul(ps_bc[:], ones128[:], acc[:],
                                         start=True, stop=True)
                        rec = miscp.tile([128, ST], F32, tag="rec")
                        nc.vector.reciprocal_approx_fast(rec[:], ps_bc[:])
                        stage = stagep.tile([128, ST], BF16, tag="stage",
                                            name=f"stage_{b}_{h}_{g}")
                        nc.vector.tensor_mul(stage[:], ps_out[:], rec[:])
                        stages[g][h] = stage
                    return emit

                def run_all(tiles, hooks):
                    stages = {b: [[None] * NH_LOC for _ in range(GP)]
                              for b in range(B)}
                    steps = _riffle(PAIRS)
                    assert len(steps) == 2 * NH_LOC * 40
                    last_of = {}
                    seen = set()
                    for idx in range(len(steps) - 1, -1, -1):
                        b, h, g, j, nj = steps[idx]
                        if (b, h, g) not in seen:
                            seen.add((b, h, g))
                            if j == nj - 1:
                                last_of[idx] = (b, h, g)
                    g_seen = {(b, g): 0 for b in range(B) for g in range(GP)}
                    grp = {}
                    probs_of = {}

                    def emit_scores(i):
                        b, h, g, j, nj = steps[i]
                        qt, kt, v_kd = tiles[(b, h)]
                        q0 = g * ST
                        r = (j - 4 * g) * 128 if j >= 4 * g else 0
                        w = ST - r
                        if (b, h, g) not in grp:
                            grp[(b, h, g)] = (
                                ps_acc.tile([128, ST], F32, tag="acc",
                                            name=f"ps_out_{b}_{h}_{g}"),
                                accp.tile([128, ST], BF16, tag="pacc",
                                          name=f"acc_{b}_{h}_{g}"),
                            )
                        ps_sc = ps_sc_p.tile([128, ST], F32, tag="sc",
                                             name=f"ps_sc_{b}_{h}_{g}_{j}")
                        nc.tensor.matmul(ps_sc[:, r:], kt[:, j * 128:(j + 1) * 128],
                                         qt[:, q0 + r:q0 + ST], start=True, stop=True)
                        probs = probsp.tile([128, ST], BF16, tag="probs",
                                            name=f"probs_{b}_{h}_{g}_{j}")
                        nc.scalar.activation(probs[:, r:], ps_sc[:, r:], AF.Exp,
                                             scale=SCALE)
                        if j >= 4 * g:
                            # only the first 128 columns past the block
                            # boundary can violate causality (row < 128 so
                            # tril is all-ones beyond) - a quarter-width
                            # multiply saves ~26us of critical-path DVE
                            mw = min(w, 128)
                            nc.vector.tensor_mul(
                                probs[:, r:r + mw], probs[:, r:r + mw],
                                mask_sb[:, 0:mw])
                        ps_out, acc = grp[(b, h, g)]
                        if j == 0:
                            nc.vector.tensor_copy(acc[:], probs[:])
                        else:
                            nc.vector.tensor_add(acc[:, r:], acc[:, r:], probs[:, r:])
                        probs_of[i] = (probs, r)

                    def emit_pv(i):
                        b, h, g, j, nj = steps[i]
                        qt, kt, v_kd = tiles[(b, h)]
                        probs, r = probs_of.pop(i)
                        ps_out, acc = grp[(b, h, g)]
                        nc.tensor.matmul(ps_out[:, r:], v_kd[:, j], probs[:, r:],
                                         start=(j == 0), stop=(j == nj - 1))
                        if i in last_of:
                            norms.append((make_norm(b, h, g, ps_out, acc, stages[b]),
                                          gstep[0]))
                            g_seen[(b, g)] += 1
                            if g_seen[(b, g)] == NH_LOC:
                                t = b * GP + g
                                for m in range(H // 128):
                                    fills.append((make_oproj_chunk(t, m, stages[b][g]),
                                                  gstep[0]))

                    # 3-deep scores pipeline: scores(i+3)'s PSUM-slot reuse
                    # waits exp(i), which is strictly weaker than pv(i)'s own
                    # probs dependency right behind it in the PE queue - so
                    # the extra depth adds exp->mask->acc slack with no new
                    # head-of-line risk.
                    LA = 3
                    for i in range(LA):
                        emit_scores(i)
                    # warmup filler: keep PE (and its HAM clock) busy for the
                    # ~2.5us the first exp->mask chain needs to fill
                    ps_w = ps_acc.tile([128, ST], F32, tag="acc", name="ps_warm")
                    for _ in range(9):
                        nc.tensor.matmul(ps_w[:], ones128[:], cs_sb[:, 0:ST],
                                         start=True, stop=True)
                    for i in range(len(steps)):
                        for fn in hooks.get(i, ()):
                            fn()
                        if i + LA < len(steps):
                            emit_scores(i + LA)
                        emit_pv(i)
                        # pop side work: norms queued >=2 steps ago first
                        # (gives the DVE acc chain time), then o_proj chunks.
                        gstep[0] += 1
                        cur = gstep[0]
                        while norms and cur > norms[0][1] + 1:
                            norms.popleft()[0]()
                        # keep a few fills in reserve near the end so the PE
                        # bridges the last group's norm latency instead of
                        # idling (which also HAM-gates the clock for the
                        # final o_proj drain)
                        reserve = 4 if cur < len(steps) - 8 else 0
                        npop = 2 if len(fills) > 16 else 1
                        for _ in range(npop):
                            if len(fills) > reserve and cur > fills[0][1] + 1:
                                fills.popleft()[0]()

                # heads (0,0)/(0,2) roped in phase 1; the rest load + rope
                # during attention via step hooks: q/k/swap loads on GPSIMD,
                # ropes on DVE (half-width), v_kd on sync.
                tiles = {(0, 0): tiles0[0], (0, 2): tiles0[2]}
                staged = {}
                hooks = {}

                def hook(step, fn):
                    hooks.setdefault(step, []).append(fn)

                def add_head(b, h, ld_step, rope_start, vkd_step):
                    holder = [None, None, None]
                    tiles[(b, h)] = holder

                    def ld():
                        qt, kt = emit_qk_load(b, h, headp)
                        sw = emit_swaps(b, h, xatt)
                        staged[(b, h)] = (qt, kt) + sw
                        holder[0], holder[1] = qt, kt
                    hook(ld_step, ld)

                    def ldv():
                        holder[2] = emit_vkd(b, h, headp, bufs=7)
                    hook(vkd_step, ldv)

                    st = {}

                    def mk(k):
                        def fn():
                            if k == 0:
                                st['ops'] = rope_ops_half(*staged[(b, h)])
                            st['ops'][k]()
                        return fn
                    for k in range(12):
                        hook(rope_start + 2 * k, mk(k))

                # first uses: (0,1)@40 (0,2)@52 (0,3)@68 (1,0)@133
                # (1,1)@134 (1,2)@153 (1,3)@154; each head's 12 half-rope
                # ops run one per two steps, stretches never overlap, and
                # each finishes >=4 steps before its first scores emission.
                hook(1, lambda: tiles[(0, 2)].__setitem__(
                    2, emit_vkd(0, 2, headp, bufs=7)))
                # h1's ropes wait until its loads have landed (~6us of DMA):
                # an earlier rope op would head-of-line-block the whole DVE
                # queue (masks/accs of the first steps) behind the load wait
                add_head(0, 1, 1, 12, 2)
                add_head(0, 3, 39, 41, 40)
                add_head(1, 0, 81, 83, 82)
                add_head(1, 1, 95, 97, 96)
                add_head(1, 2, 109, 111, 110)
                add_head(1, 3, 123, 125, 124)
                for q in range(4):
                    hook(44 + 10 * q, emit_wo_load(q))
                run_all(tiles, hooks)

                while norms:
                    norms.popleft()[0]()
                while fills:
                    fills.popleft()[0]()

    nc.finalize()
    return nc


def _prep_inputs(positions, hidden_states, w_pack, w_o):
    pos = np.asarray(positions).astype(np.float32)
    hid = np.asarray(hidden_states, dtype=np.float32)
    w_pack = np.asarray(w_pack, dtype=np.float32)
    w_o = np.asarray(w_o, dtype=np.float32)

    hT = np.ascontiguousarray(hid.reshape(BS, H).T).astype(BF)

    inv_freq = 1.0 / (ROPE_THETA ** (np.arange(0, D, 2, dtype=np.float32) / D))
    ang = pos[None, :] * inv_freq[:, None]              # [64, S]
    cos = np.cos(ang).astype(np.float32)
    sin = np.sin(ang).astype(np.float32)
    cs = np.ascontiguousarray(np.concatenate([cos, cos], 0)).astype(BF)    # [128, S]
    sn = np.ascontiguousarray(np.concatenate([-sin, sin], 0)).astype(BF)

    mask = (np.arange(ST)[None, :] >= np.arange(128)[:, None]).astype(BF)  # [128, 512]

    in_maps = []
    for c in range(NCORES):
        j0 = 512 * c
        w1 = np.concatenate([w_pack[:, j0:j0 + 512],
                             w_pack[:, H + j0:H + j0 + 512]], axis=1)
        # pack to the SBUF layout [p, m, ko, col]: w1p[p, m, ko, c] = w1[ko*128+p, m*128+c]
        w1p = np.ascontiguousarray(
            w1.reshape(HK, 128, M_QK, 128).transpose(1, 2, 0, 3)).astype(BF)
        # v weights stay [p, ko, col] (rhs of the token-major vT chains)
        wvp = np.ascontiguousarray(
            w_pack[:, 2 * H + j0:2 * H + j0 + 512].reshape(HK, 128, 512)
            .transpose(1, 0, 2)).astype(BF)
        wo = np.ascontiguousarray(w_o[j0:j0 + 512, :]).astype(BF)
        in_maps.append({
            "hT": hT, "w1": w1p, "wv": wvp, "wo": wo,
            "cs": cs, "sn": sn, "mask": mask,
        })
    return in_maps


def kernel(positions, hidden_states, w_pack, w_o):
    global LAST_RESULT
    nc = _build_program()
    in_maps = _prep_inputs(positions, hidden_states, w_pack, w_o)
    res = run_bass_kernel_spmd(
        nc, in_maps, core_ids=list(range(NCORES)),
        trace=bool(os.environ.get("BASS_TRACE")))
    LAST_RESULT = res
    acc = np.zeros((H, BS), np.float32)
    for r in res.results:
        acc += r["out"].astype(np.float32)
    return np.ascontiguousarray(acc.T).reshape(B, S, H)


# revision 36
# speedup vs baseline: 1.0021x; 1.0021x over previous
"""BaiChuan attention block (QKV proj + RoPE + causal attention + o_proj) on 8 NeuronCores.

Sharding: tensor-parallel over heads. Each core owns 4 of the 32 heads:
W_pack columns (q/k/v slices) are column-sharded, w_o is row-sharded, and the
8 partial o_proj outputs are summed on the host (cheap f32 reduce) instead of
an on-device all-reduce.

Everything on-device runs in bf16 (fp32 PSUM accumulation). q/k activations
are kept feature-major ("transposed", [feature, batch*seq]) so that softmax
sums run along the PSUM partition axis:
  scoresT[k, q] = K_chunk @ Q_group    (lhsT = KT chunk, rhs = QT group)
  probsT = exp(scoresT * scale)        (scale folded into the ACT scale
                                        operand; no max subtraction:
                                        |scores| <= ~12 for this distribution)
  causal mask  = sliding slice of a constant 0/1 tril tile, applied only to
                 the diagonal chunks, which are also column-trimmed
  outT[d, q]  += V_kd chunk @ probsT   (PSUM accumulate over k chunks)
  denom+bcast  = ones[128x128] @ acc   (one matmul reduces acc over k AND
                                        broadcasts the denominator)
  normalize    = reciprocal_approx_fast + multiply on DVE

V is produced TOKEN-major in phase 1 (vT chains: lhsT = ht tok-chunk, rhs =
w_v columns) so the attention PV matmul's lhsT tiles ([k-tok, d]) load with a
plain contiguous DMA - no DmaTranspose, whose multi-us sequencer hold was
blocking the store queue at the batch seam in the previous version.

Engine assignment: TensorE matmuls (incl. dependency-free warmup fillers that
hold the DVFS clock up through the DMA-gated startup); ScalarE = exp +
PSUM->SBUF copies + vT stores; DVE = masks (quarter-width: the tril is
all-ones past col 128), prob accumulation, normalize, and ropes (half-width
ops, scheduled only where the head's loads have already landed, so a rope op
never head-of-line-blocks the mask/acc chain that gates PV matmuls); GPSIMD =
phase-1 head loads only (its SWDGE is too slow for the attention-phase bursts
and its tensor ops steal the DVE's SBUF port); sync HWDGE = phase-1 w/ht
loads + q/k stores, attention head loads, wo loads, v_kd loads, o_proj
stores.

Scheduling (engine streams are static and in-order, so overlap is baked into
emission order):
  - t=0 of the qkv projection runs 6 m-chains interleaved by ko-octs so the
    first matmul is gated by ~1.25MB of DMA, not ~5MB, and the PE has more
    standing work per oct than the contended startup DMA needs to deliver.
  - heads (0,0) and (0,2) load+rope DURING phase 1 (idle DVE), into pools
    allocated before the phase-1 pools so addresses never conflict.
  - both batches run as ONE continuous riffle of (b,h,g) j-streams; b=1
    streams weave in as the b=0 streams retire, so there is no batch seam
    where the PE runs dry. A 3-deep scores pipeline keeps >=6 independent
    matmuls between a scores matmul and the PV matmul that needs its probs.
  - o_proj m-chunks and denominator matmuls are queued side work, popped
    after each PV step (denominators with priority: PSUM/acc rings recycle
    only after a group's normalize; a 4-chunk reserve bridges the last
    group's normalize latency so the final drain starts at full clock).
"""

import os
from collections import deque
import numpy as np
import ml_dtypes

import concourse.bass as bass
import concourse.tile as tile
import concourse.mybir as mybir
from concourse import bacc
from concourse.bass_utils import run_bass_kernel_spmd

F32 = mybir.dt.float32
BF16 = mybir.dt.bfloat16
AF = mybir.ActivationFunctionType
BF = ml_dtypes.bfloat16

B, S, H = 2, 2048, 4096
BS = B * S                      # 4096 tokens
D = 128                         # head dim
NCORES = 8
NH_LOC = 4                      # heads per core (32 / 8)
HK = H // 128                   # 32 contraction chunks for qkv proj
M_QK = 2 * NH_LOC               # 8 q/k output row-chunks per core
ST = 512                        # seq tile
NT = BS // ST                   # 8 seq tiles
GP = S // ST                    # 4 q-groups per sequence
SC = S // 128                   # 16 k-chunks per sequence
ROPE_THETA = 10000.0
SCALE = D ** -0.5

# One continuous riffle across BOTH batches: groups of (b, h, g) j-streams
# interleaved. b=1 streams enter as b=0 streams retire (their qkv rows are
# only stored at the end of phase 1), ending group-major so only the last
# group's 32 o_proj chunks drain after the last pv step.
PAIRS = [
    [(0, 0, 1), (0, 0, 2)],
    [(0, 0, 0), (0, 0, 3)],
    [(0, 1, 0), (0, 1, 1)],
    [(0, 2, 0), (0, 1, 2)],
    [(0, 3, 0), (0, 2, 1)],
    [(0, 1, 3), (0, 2, 2)],
    [(0, 3, 1), (0, 2, 3)],
    [(0, 3, 2), (1, 0, 0), (1, 1, 0)],
    [(0, 3, 3), (1, 2, 0), (1, 3, 0)],
    [(1, 0, 1), (1, 1, 1)],
    [(1, 2, 1), (1, 3, 1)],
    [(1, 0, 2), (1, 1, 2)],
    [(1, 2, 2), (1, 3, 2)],
    [(1, 0, 3), (1, 1, 3)],
    [(1, 2, 3), (1, 3, 3)],
]

LAST_RESULT = None              # BassKernelResults of the most recent run (for test.py)


def _riffle(pairs):
    """[(b,h,g)...] stream groups -> flat (b, h, g, j, nj) step list."""
    steps = []
    for grp_list in pairs:
        streams = [[(b, h, g, j, 4 * g + 4) for j in range(4 * g + 4)]
                   for b, h, g in grp_list]
        k = 0
        while any(streams):
            st = streams[k % len(streams)]
            if st:
                steps.append(st.pop(0))
            k += 1
    return steps


def _build_program():
    nc = bacc.Bacc()

    hT = nc.dram_tensor("hT", [H, BS], BF16, kind="ExternalInput")
    w1 = nc.dram_tensor("w1", [128, M_QK, HK, 128], BF16, kind="ExternalInput")
    wv = nc.dram_tensor("wv", [128, HK, 512], BF16, kind="ExternalInput")
    wo = nc.dram_tensor("wo", [NH_LOC * 128, H], BF16, kind="ExternalInput")
    cs = nc.dram_tensor("cs", [128, S], BF16, kind="ExternalInput")
    sn = nc.dram_tensor("sn", [128, S], BF16, kind="ExternalInput")
    maskd = nc.dram_tensor("mask", [128, ST], BF16, kind="ExternalInput")
    out = nc.dram_tensor("out", [H, BS], BF16, kind="ExternalOutput")

    with tile.TileContext(nc) as tc:
        with (
            tc.tile_pool(name="cons", bufs=1) as cons,
            tc.tile_pool(name="dram", bufs=1, space="DRAM") as dram,
            tc.tile_pool(name="ps_acc", bufs=5, space="PSUM") as ps_acc,
            tc.tile_pool(name="ps_sc", bufs=3, space="PSUM") as ps_sc_p,
            tc.tile_pool(name="xload", bufs=1) as xload,
            tc.tile_pool(name="eheads", bufs=1) as eheads,
        ):
            # per-(row-chunk, batch) bounce tiles: a head's read then depends
            # only on the writes that filled its own tile, not on the whole
            # phase-1 write stream.
            qkv_t = [[dram.tile([128, S], BF16, name=f"qkv_{m}_{bb}",
                                tag=f"qkv_{m}_{bb}")
                      for bb in range(B)] for m in range(M_QK)]
            # token-major V bounce: [tok-in-chunk, k-chunk, d] per (b, head)
            vtb = [[dram.tile([128, SC, 128], BF16, name=f"vt_{bb}_{h}",
                              tag=f"vt_{bb}_{h}")
                    for h in range(NH_LOC)] for bb in range(B)]
            hT3 = hT.rearrange("(ko p) s -> p ko s", p=128)

            def emit_qk_load(b, h, pool, tags=("qt", "kt"), eng=None):
                """Load raw q/k straight into the rope-target tiles (rope
                then runs in place). GPSIMD SWDGE moves ~0.5MB per 2us with
                serialized transfers, so attention-phase heads load on the
                fast sync HWDGE (a 6-issue burst costs the store stream
                <4us of queue, within the ob ring's slack); phase-1 heads
                use the then-idle gpsimd queue."""
                eng = eng or nc.sync
                nm = f"{b}_{h}"
                qt = pool.tile([128, S], BF16, tag=tags[0], name=f"qt_{nm}")
                eng.dma_start(qt[:], qkv_t[h][b][:])
                kt = pool.tile([128, S], BF16, tag=tags[1], name=f"kt_{nm}")
                eng.dma_start(kt[:], qkv_t[NH_LOC + h][b][:])
                return qt, kt

            def emit_swaps(b, h, pool, eng=None):
                """Partition-swapped q/k copies for the rope sin term."""
                eng = eng or nc.sync
                xqs = pool.tile([128, S], BF16, tag="xqs", name=f"xqs_{b}_{h}")
                eng.dma_start(xqs[0:64, :], qkv_t[h][b][64:128, :])
                eng.dma_start(xqs[64:128, :], qkv_t[h][b][0:64, :])
                xks = pool.tile([128, S], BF16, tag="xks", name=f"xks_{b}_{h}")
                eng.dma_start(xks[0:64, :], qkv_t[NH_LOC + h][b][64:128, :])
                eng.dma_start(xks[64:128, :], qkv_t[NH_LOC + h][b][0:64, :])
                return xqs, xks

            def emit_vkd(b, h, pool, tag="vkd", bufs=None):
                """Plain contiguous load of the token-major V tile."""
                v_kd = pool.tile([128, SC, 128], BF16, tag=tag, bufs=bufs,
                                 name=f"vkd_{b}_{h}")
                nc.sync.dma_start(v_kd[:], vtb[b][h][:])
                return v_kd

            def rope_ops_dve(qt, kt, xqs, xks):
                """Six full-width in-place rope emitters for the idle-DVE
                phase-1 heads."""
                return [
                    lambda: nc.vector.tensor_mul(qt[:], qt[:], cs_sb[:]),
                    lambda: nc.vector.tensor_mul(xqs[:], xqs[:], sn_sb[:]),
                    lambda: nc.vector.tensor_add(qt[:], qt[:], xqs[:]),
                    lambda: nc.vector.tensor_mul(kt[:], kt[:], cs_sb[:]),
                    lambda: nc.vector.tensor_mul(xks[:], xks[:], sn_sb[:]),
                    lambda: nc.vector.tensor_add(kt[:], kt[:], xks[:]),
                ]

            def rope_ops_half(qt, kt, xqs, xks):
                """Attention-phase rope on DVE in half-width ops so a rope
                op ahead of a mask/acc op delays it by <=0.7us. (GPSIMD
                tensor ops are NOT an alternative: they steal the SBUF port
                they share with DVE and slow concurrent DVE ops ~10x.)"""
                hs = [slice(0, S // 2), slice(S // 2, S)]
                ops = []
                for dst, src in ((qt, xqs), (kt, xks)):
                    for hsl in hs:
                        ops.append(lambda d=dst, h=hsl: nc.vector.tensor_mul(
                            d[:, h], d[:, h], cs_sb[:, h]))
                    for hsl in hs:
                        ops.append(lambda x=src, h=hsl: nc.vector.tensor_mul(
                            x[:, h], x[:, h], sn_sb[:, h]))
                    for hsl in hs:
                        ops.append(lambda d=dst, x=src, h=hsl: nc.vector.tensor_add(
                            d[:, h], d[:, h], x[:, h]))
                return ops

            # ---------------- Phase 1: qkT = w1.T @ hT ; vT = hT.T @ wv ----------------
            with (
                tc.tile_pool(name="w1p", bufs=1) as w1p,
                tc.tile_pool(name="wvp", bufs=1) as wvp,
                tc.tile_pool(name="htp", bufs=2) as htp,
                tc.tile_pool(name="p1o", bufs=4) as p1o,
            ):
                w_sb = w1p.tile([128, M_QK, HK, 128], BF16, tag="w1")
                wv_sb = wvp.tile([128, HK, 512], BF16, tag="wv")

                # PE clock warmup: the first ~15us are DMA-gated, and the PE
                # p-state only reaches max clock after ~3us of continuous
                # execution. Grind dependency-free matmuls on memset tiles so
                # pass-A starts at full clock instead of ramping through it.
                ones128 = cons.tile([128, 128], BF16, tag="ones128")
                nc.vector.memset(ones128[:], 1.0)
                wm_sb = cons.tile([128, ST], BF16, tag="wm")
                nc.vector.memset(wm_sb[:], 0.0)
                ps_w0 = ps_acc.tile([128, ST], F32, tag="acc", name="ps_warm0")
                for _ in range(12):
                    nc.tensor.matmul(ps_w0[:], ones128[:], wm_sb[:],
                                     start=True, stop=True)

                def emit_ht(t):
                    ht = htp.tile([128, HK, ST], BF16, tag="ht", name=f"ht_{t}")
                    for oct_ in range(4):
                        nc.sync.dma_start(
                            ht[:, oct_ * 8:(oct_ + 1) * 8],
                            hT3[:, oct_ * 8:(oct_ + 1) * 8, t * ST:(t + 1) * ST])
                    return ht

                # Startup: pass-A (t=0, m=0..5) runs ko-oct-interleaved, so
                # ship w/ht in matching oct-sized pieces: the first matmul is
                # gated by w(m0,oct0)+ht(oct0) = 1.25MB, and each later oct's
                # pieces arrive while the PE chews the previous oct. Nothing
                # else (constants included) is emitted ahead of this prefix.
                # 6 concurrent chains (5 acc slots + 1 borrowed from the
                # idle scores ring) give ~10.4us of PE work per oct vs
                # ~7.5us of contended startup DMA - the PE stays ahead.
                NA = 6
                nc.sync.dma_start(w_sb[:, 0, 0:8], w1[:, 0, 0:8])
                ht0 = htp.tile([128, HK, ST], BF16, tag="ht", name="ht_0")
                nc.sync.dma_start(ht0[:, 0:8], hT3[:, 0:8, 0:ST])
                nc.sync.dma_start(w_sb[:, 1:NA, 0:8], w1[:, 1:NA, 0:8])
                for oct_ in range(1, 4):
                    nc.sync.dma_start(ht0[:, oct_ * 8:(oct_ + 1) * 8],
                                      hT3[:, oct_ * 8:(oct_ + 1) * 8, 0:ST])
                    nc.sync.dma_start(w_sb[:, 0:NA, oct_ * 8:(oct_ + 1) * 8],
                                      w1[:, 0:NA, oct_ * 8:(oct_ + 1) * 8])
                nc.sync.dma_start(w_sb[:, NA:M_QK], w1[:, NA:M_QK])
                for oct_ in range(4):
                    nc.sync.dma_start(wv_sb[:, oct_ * 8:(oct_ + 1) * 8],
                                      wv[:, oct_ * 8:(oct_ + 1) * 8])

                # constants ride behind the startup-critical weights: first
                # needed by the t==5 rope hook / first attention scores.
                cs_sb = cons.tile([128, S], BF16, tag="cs")
                nc.scalar.dma_start(cs_sb[:], cs[:])
                sn_sb = cons.tile([128, S], BF16, tag="sn")
                nc.scalar.dma_start(sn_sb[:], sn[:])
                mask_sb = cons.tile([128, ST], BF16, tag="mask")
                nc.scalar.dma_start(mask_sb[:], maskd[:])

                def qk_store(t, m, ob):
                    tc_, tb = t % GP, t // GP
                    # t=7 stores issue from the ACT queue (woven between its
                    # own copies, done pre-transition) so the first b=1 head
                    # loads aren't FIFO-blocked behind them
                    eng = nc.scalar if t == NT - 1 else nc.sync
                    eng.dma_start(qkv_t[m][tb][:, tc_ * ST:(tc_ + 1) * ST], ob[:])

                estaged = {}
                tiles0 = {}
                ht_next = ht0
                for t in range(NT):
                    ht = ht_next
                    if t < NT - 1:
                        # prefetch emission: ht(t+1) enters the sync FIFO
                        # ahead of tile t's qkv stores
                        ht_next = emit_ht(t + 1)
                    ms = list(range(M_QK))
                    if t == 0:
                        psA = [ps_acc.tile([128, ST], F32, tag="acc",
                                           name=f"ps_q_0_{m}") for m in range(5)]
                        psA.append(ps_sc_p.tile([128, ST], F32, tag="sc",
                                                name="ps_q_0_5"))
                        for oct_ in range(4):
                            for m in range(NA):
                                for ko in range(oct_ * 8, oct_ * 8 + 8):
                                    nc.tensor.matmul(
                                        psA[m][:], w_sb[:, m, ko], ht[:, ko],
                                        start=(ko == 0), stop=(ko == HK - 1))
                            if oct_ < 3:
                                # dependency-free fillers bridge the tail of
                                # the next oct's DMA so the clock never gates
                                for _ in range(3):
                                    nc.tensor.matmul(ps_w0[:], ones128[:],
                                                     wm_sb[:], start=True,
                                                     stop=True)
                        for m in range(NA):
                            ob = p1o.tile([128, ST], BF16, tag="ob")
                            nc.scalar.activation(ob[:], psA[m][:], AF.Copy)
                            qk_store(t, m, ob)
                        ms = list(range(NA, M_QK))
                    for m in ms:
                        ps = ps_acc.tile([128, ST], F32, tag="acc",
                                         name=f"ps_q_{t}_{m}")
                        for ko in range(HK):
                            nc.tensor.matmul(
                                ps[:], w_sb[:, m, ko], ht[:, ko],
                                start=(ko == 0), stop=(ko == HK - 1))
                        ob = p1o.tile([128, ST], BF16, tag="ob")
                        nc.scalar.activation(ob[:], ps[:], AF.Copy)
                        qk_store(t, m, ob)
                    # token-major V: out[tok, d] accumulated over ko; lhsT is
                    # the ht token-chunk itself - no transpose anywhere.
                    tc_, tb = t % GP, t // GP
                    for c in range(4):
                        ps = ps_acc.tile([128, ST], F32, tag="acc",
                                         name=f"ps_v_{t}_{c}")
                        for ko in range(HK):
                            nc.tensor.matmul(
                                ps[:], ht[:, ko, c * 128:(c + 1) * 128],
                                wv_sb[:, ko],
                                start=(ko == 0), stop=(ko == HK - 1))
                        obv = p1o.tile([128, ST], BF16, tag="obv")
                        nc.scalar.activation(obv[:], ps[:], AF.Copy)
                        for h in range(NH_LOC):
                            nc.scalar.dma_start(
                                vtb[tb][h][:, tc_ * 4 + c, :],
                                obv[:, h * 128:(h + 1) * 128])
                    if t == 3:
                        # heads (0,0)/(0,2) raw q/k + swaps; their ropes run
                        # on the idle DVE at t=5/6 so early attention carries
                        # only h1/h3/b1 rope work
                        qt0, kt0 = emit_qk_load(0, 0, eheads, tags=("qt0", "kt0"),
                                                eng=nc.gpsimd)
                        sw0 = emit_swaps(0, 0, xload, eng=nc.gpsimd)
                        estaged[0] = (qt0, kt0) + sw0
                    elif t == 4:
                        qt2, kt2 = emit_qk_load(0, 2, eheads, tags=("qt2", "kt2"),
                                                eng=nc.gpsimd)
                        # xload ring is 1-deep: h2's swap DMAs queue on
                        # gpsimd behind the WAR on h0's t=5 rope reads
                        sw2 = emit_swaps(0, 2, xload, eng=nc.gpsimd)
                        estaged[2] = (qt2, kt2) + sw2
                    elif t == 5:
                        for op in rope_ops_dve(*estaged[0]):
                            op()
                        tiles0[0] = (estaged[0][0], estaged[0][1],
                                     emit_vkd(0, 0, eheads, tag="vk0"))
                    elif t == 6:
                        for op in rope_ops_dve(*estaged[2]):
                            op()
                        tiles0[2] = [estaged[2][0], estaged[2][1], None]

            # ---------------- Phase 2+3: attention with interleaved o_proj ----------------
            with (
                tc.tile_pool(name="headp", bufs=6) as headp,
            tc.tile_pool(name="xatt", bufs=2) as xatt,
                tc.tile_pool(name="probsp", bufs=6) as probsp,
                tc.tile_pool(name="accp", bufs=5) as accp,
                tc.tile_pool(name="stagep", bufs=22) as stagep,
                tc.tile_pool(name="miscp", bufs=2) as miscp,
                tc.tile_pool(name="p3w", bufs=1) as wop,
                tc.tile_pool(name="p3o", bufs=6) as p3o,
            ):
                wo_sb = wop.tile([128, NH_LOC, H], BF16, tag="wo")
                wo3 = wo.rearrange("(ko p) f -> p ko f", p=128)

                def emit_wo_load(q):
                    # quarter loads deferred into hooks so head loads aren't
                    # queued behind 4.2MB of weights; quarter q covers o_proj
                    # chunks m in [8q, 8q+8)
                    def fn():
                        c0 = q * (H // 4)
                        nc.sync.dma_start(wo_sb[:, :, c0:c0 + H // 4],
                                          wo3[:, :, c0:c0 + H // 4])
                    return fn

                # PE side-work queues. norms (denominator matmuls) must never
                # starve behind o_proj chunks: a group's PSUM/acc slots free
                # only after its normalize runs, and the rings wrap quickly.
                norms = deque()     # entries: (emit_fn, pushed_step)
                fills = deque()
                gstep = [0]

                def make_oproj_chunk(t, m, stages):
                    def emit():
                        ps = ps_acc.tile([128, ST], F32, tag="acc", name=f"ps_o_{t}_{m}")
                        for ko in range(NH_LOC):
                            nc.tensor.matmul(
                                ps[:], wo_sb[:, ko, m * 128:(m + 1) * 128],
                                stages[ko][:],
                                start=(ko == 0), stop=(ko == NH_LOC - 1))
                        ob = p3o.tile([128, ST], BF16, tag="ob3", name=f"ob3_{t}_{m}")
                        # copies stay on ScalarE: a measured DVE/ACT split
                        # rebalanced busy% but WORSENED PE idle - DVE (masks/
                        # accs/ropes) is the critical engine, ScalarE is not
                        nc.scalar.activation(ob[:], ps[:], AF.Copy)
                        nc.sync.dma_start(
                            out[m * 128:(m + 1) * 128, t * ST:(t + 1) * ST], ob[:])
                    return emit

                def make_norm(b, h, g, ps_out, acc, stages):
                    def emit():
                        # denominator broadcast rides the "acc" ring so the
                        # scores ring keeps all 3 slots for the 2-deep
                        # scores pipeline
                        ps_bc = ps_acc.tile([128, ST], F32, tag="acc", name=f"ps_bc_{b}_{h}_{g}")
                        nc.tensor.matmul(ps_bc[:], ones128[:], acc[:],
                                         start=True, stop=True)
                        rec = miscp.tile([128, ST], F32, tag="rec")
                        nc.vector.reciprocal_approx_fast(rec[:], ps_bc[:])
                        stage = stagep.tile([128, ST], BF16, tag="stage",
                                            name=f"stage_{b}_{h}_{g}")
                        nc.vector.tensor_mul(stage[:], ps_out[:], rec[:])
                        stages[g][h] = stage
                    return emit

                def run_all(tiles, hooks):
                    stages = {b: [[None] * NH_LOC for _ in range(GP)]
                              for b in range(B)}
                    steps = _riffle(PAIRS)
                    assert len(steps) == 2 * NH_LOC * 40
                    last_of = {}
                    seen = set()
                    for idx in range(len(steps) - 1, -1, -1):
                        b, h, g, j, nj = steps[idx]
                        if (b, h, g) not in seen:
                            seen.add((b, h, g))
                            if j == nj - 1:
                                last_of[idx] = (b, h, g)
                    g_seen = {(b, g): 0 for b in range(B) for g in range(GP)}
                    grp = {}
                    probs_of = {}

                    def emit_scores(i):
                        b, h, g, j, nj = steps[i]
                        qt, kt, v_kd = tiles[(b, h)]
                        q0 = g * ST
                        r = (j - 4 * g) * 128 if j >= 4 * g else 0
                        w = ST - r
                        if (b, h, g) not in grp:
                            grp[(b, h, g)] = (
                                ps_acc.tile([128, ST], F32, tag="acc",
                                            name=f"ps_out_{b}_{h}_{g}"),
                                accp.tile([128, ST], BF16, tag="pacc",
                                          name=f"acc_{b}_{h}_{g}"),
                            )
                        ps_sc = ps_sc_p.tile([128, ST], F32, tag="sc",
                                             name=f"ps_sc_{b}_{h}_{g}_{j}")
                        nc.tensor.matmul(ps_sc[:, r:], kt[:, j * 128:(j + 1) * 128],
                                         qt[:, q0 + r:q0 + ST], start=True, stop=True)
                        probs = probsp.tile([128, ST], BF16, tag="probs",
                                            name=f"probs_{b}_{h}_{g}_{j}")
                        nc.scalar.activation(probs[:, r:], ps_sc[:, r:], AF.Exp,
                                             scale=SCALE)
                        if j >= 4 * g:
                            # only the first 128 columns past the block
                            # boundary can violate causality (row < 128 so
                            # tril is all-ones beyond) - a quarter-width
                            # multiply saves ~26us of critical-path DVE
                            mw = min(w, 128)
                            nc.vector.tensor_mul(
                                probs[:, r:r + mw], probs[:, r:r + mw],
                                mask_sb[:, 0:mw])
                        ps_out, acc = grp[(b, h, g)]
                        if j == 0:
                            nc.vector.tensor_copy(acc[:], probs[:])
                        else:
                            nc.vector.tensor_add(acc[:, r:], acc[:, r:], probs[:, r:])
                        probs_of[i] = (probs, r)

                    def emit_pv(i):
                        b, h, g, j, nj = steps[i]
                        qt, kt, v_kd = tiles[(b, h)]
                        probs, r = probs_of.pop(i)
                        ps_out, acc = grp[(b, h, g)]
                        nc.tensor.matmul(ps_out[:, r:], v_kd[:, j], probs[:, r:],
                                         start=(j == 0), stop=(j == nj - 1))
                        if i in last_of:
                            norms.append((make_norm(b, h, g, ps_out, acc, stages[b]),
                                          gstep[0]))
                            g_seen[(b, g)] += 1
                            if g_seen[(b, g)] == NH_LOC:
                                t = b * GP + g
                                for m in range(H // 128):
                                    fills.append((make_oproj_chunk(t, m, stages[b][g]),
                                                  gstep[0]))

                    # 3-deep scores pipeline: scores(i+3)'s PSUM-slot reuse
                    # waits exp(i), which is strictly weaker than pv(i)'s own
                    # probs dependency right behind it in the PE queue - so
                    # the extra depth adds exp->mask->acc slack with no new
                    # head-of-line risk.
                    LA = 3
                    for i in range(LA):
                        emit_scores(i)
                    # warmup filler: keep PE (and its HAM clock) busy for the
                    # ~2.5us the first exp->mask chain needs to fill
                    ps_w = ps_acc.tile([128, ST], F32, tag="acc", name="ps_warm")
                    for _ in range(9):
                        nc.tensor.matmul(ps_w[:], ones128[:], cs_sb[:, 0:ST],
                                         start=True, stop=True)
                    for i in range(len(steps)):
                        for fn in hooks.get(i, ()):
                            fn()
                        if i + LA < len(steps):
                            emit_scores(i + LA)
                        emit_pv(i)
                        # pop side work: norms queued >=2 steps ago first
                        # (gives the DVE acc chain time), then o_proj chunks.
                        gstep[0] += 1
                        cur = gstep[0]
                        while norms and cur > norms[0][1] + 1:
                            norms.popleft()[0]()
                        # keep a few fills in reserve near the end so the PE
                        # bridges the last group's norm latency instead of
                        # idling (which also HAM-gates the clock for the
                        # final o_proj drain)
                        reserve = 4 if cur < len(steps) - 8 else 0
                        npop = 2 if len(fills) > 16 else 1
                        for _ in range(npop):
                            if len(fills) > reserve and cur > fills[0][1] + 1:
                                fills.popleft()[0]()

                # heads (0,0)/(0,2) roped in phase 1; the rest load + rope
                # during attention via step hooks: q/k/swap loads on GPSIMD,
                # ropes on DVE (half-width), v_kd on sync.
                tiles = {(0, 0): tiles0[0], (0, 2): tiles0[2]}
                staged = {}
                hooks = {}

                def hook(step, fn):
                    hooks.setdefault(step, []).append(fn)

                def add_head(b, h, ld_step, rope_start, vkd_step):
                    holder = [None, None, None]
                    tiles[(b, h)] = holder

                    def ld():
                        qt, kt = emit_qk_load(b, h, headp)
                        sw = emit_swaps(b, h, xatt)
                        staged[(b, h)] = (qt, kt) + sw
                        holder[0], holder[1] = qt, kt
                    hook(ld_step, ld)

                    def ldv():
                        holder[2] = emit_vkd(b, h, headp, bufs=7)
                    hook(vkd_step, ldv)

                    st = {}

                    def mk(k):
                        def fn():
                            if k == 0:
                                st['ops'] = rope_ops_half(*staged[(b, h)])
                            st['ops'][k]()
                        return fn
                    for k in range(12):
                        hook(rope_start + 2 * k, mk(k))

                # first uses: (0,1)@40 (0,2)@52 (0,3)@68 (1,0)@133
                # (1,1)@134 (1,2)@153 (1,3)@154; each head's 12 half-rope
                # ops run one per two steps, stretches never overlap, and
                # each finishes >=4 steps before its first scores emission.
                hook(1, lambda: tiles[(0, 2)].__setitem__(
                    2, emit_vkd(0, 2, headp, bufs=7)))
                # h1's ropes wait until its loads have landed (~6us of DMA):
                # an earlier rope op would head-of-line-block the whole DVE
                # queue (masks/accs of the first steps) behind the load wait
                add_head(0, 1, 1, 8, 2)
                add_head(0, 3, 33, 37, 34)
                add_head(1, 0, 79, 83, 80)
                add_head(1, 1, 93, 97, 94)
                add_head(1, 2, 107, 111, 108)
                add_head(1, 3, 121, 125, 122)
                for q in range(4):
                    hook(44 + 10 * q, emit_wo_load(q))
                run_all(tiles, hooks)

                while norms:
                    norms.popleft()[0]()
                while fills:
                    fills.popleft()[0]()

    nc.finalize()
    return nc


def _prep_inputs(positions, hidden_states, w_pack, w_o):
    pos = np.asarray(positions).astype(np.float32)
    hid = np.asarray(hidden_states, dtype=np.float32)
    w_pack = np.asarray(w_pack, dtype=np.float32)
    w_o = np.asarray(w_o, dtype=np.float32)

    hT = np.ascontiguousarray(hid.reshape(BS, H).T).astype(BF)

    inv_freq = 1.0 / (ROPE_THETA ** (np.arange(0, D, 2, dtype=np.float32) / D))
    ang = pos[None, :] * inv_freq[:, None]              # [64, S]
    cos = np.cos(ang).astype(np.float32)
    sin = np.sin(ang).astype(np.float32)
    cs = np.ascontiguousarray(np.concatenate([cos, cos], 0)).astype(BF)    # [128, S]
    sn = np.ascontiguousarray(np.concatenate([-sin, sin], 0)).astype(BF)

    mask = (np.arange(ST)[None, :] >= np.arange(128)[:, None]).astype(BF)  # [128, 512]

    in_maps = []
    for c in range(NCORES):
        j0 = 512 * c
        w1 = np.concatenate([w_pack[:, j0:j0 + 512],
                             w_pack[:, H + j0:H + j0 + 512]], axis=1)
        # pack to the SBUF layout [p, m, ko, col]: w1p[p, m, ko, c] = w1[ko*128+p, m*128+c]
        w1p = np.ascontiguousarray(
            w1.reshape(HK, 128, M_QK, 128).transpose(1, 2, 0, 3)).astype(BF)
        # v weights stay [p, ko, col] (rhs of the token-major vT chains)
        wvp = np.ascontiguousarray(
            w_pack[:, 2 * H + j0:2 * H + j0 + 512].reshape(HK, 128, 512)
            .transpose(1, 0, 2)).astype(BF)
        wo = np.ascontiguousarray(w_o[j0:j0 + 512, :]).astype(BF)
        in_maps.append({
            "hT": hT, "w1": w1p, "wv": wvp, "wo": wo,
            "cs": cs, "sn": sn, "mask": mask,
        })
    return in_maps


def kernel(positions, hidden_states, w_pack, w_o):
    global LAST_RESULT
    nc = _build_program()
    in_maps = _prep_inputs(positions, hidden_states, w_pack, w_o)
    res = run_bass_kernel_spmd(
        nc, in_maps, core_ids=list(range(NCORES)),
        trace=bool(os.environ.get("BASS_TRACE")))
    LAST_RESULT = res
    acc = np.zeros((H, BS), np.float32)
    for r in res.results:
        acc += r["out"].astype(np.float32)
    return np.ascontiguousarray(acc.T).reshape(B, S, H)
